# revision 3
# baseline (speedup 1.0000x reference)
"""GSA (linear attention) Bass kernel for TRN2 — v2.

Per core (pure data-parallel over batch): out = W_comb @ img + b, where
W_comb folds q-proj, readout and out-proj (linear after the softmax), and
context is computed WITHOUT a v tensor:
    context_h = ek @ img^T @ W_v,h^T   with  ek = exp(W_k img)
so pass A is only: k-proj (2 MMs/tile) + exp (ACT) + G += ek @ [img^T | 1]
(4 MMs/tile, ones-column gives the softmax row-sums S for free).

dtypes: img cast to fp16 on HOST (half DMA, no on-device convert); both
layouts (channel-major for k-proj/pass B, token-major for G) are uploaded.
Output written fp16, upcast on host. Fold runs in f32/f32r.
"""
import numpy as np

HEADS, DK = 8, 64
B, C, X, Y = 8, 256, 128, 128
N_TOK = X * Y          # 16384
DH = HEADS * DK        # 512
N_CORES = 8

TA = 128               # pass A token tile
TB = 1024              # pass B token tile
CT = 272               # img_tm padded cols: 256 ch + ones col @256 + 15 pad (16B-aligned rows)
LAG = 2                # pass A software pipeline depth (k ahead of G)


def _build_program(n_tok=N_TOK, tb=TB, repeat=1, img_chunk=512,
                   psk_bufs=2, ek_bufs=3, pb_bufs=4, pso_bufs=2,
                   warmup=52, fold_warm=8):
    from contextlib import ExitStack
    import concourse.bacc as bacc
    import concourse.mybir as mybir
    import concourse.tile as tile
    from concourse import masks

    F32 = mybir.dt.float32
    F32R = mybir.dt.float32r
    F16 = mybir.dt.float16
    F8 = mybir.dt.float8e4
    DR = mybir.MatmulPerfMode.DoubleRow
    AF = mybir.ActivationFunctionType

    nta = n_tok // TA
    ntb = n_tok // tb

    nc = bacc.Bacc("TRN2", debug=False, num_devices=N_CORES)
    img_cm_d = nc.dram_tensor("img_cm", [C, n_tok], F16, kind="ExternalInput").ap() \
        .rearrange("(c2 p) n -> p c2 n", p=128)
    img_tm_d = nc.dram_tensor("img_tm", [n_tok, CT], F8, kind="ExternalInput").ap() \
        .rearrange("(t p) c -> p t c", p=128)
    wk_d = nc.dram_tensor("w_kT", [C, DH], F16, kind="ExternalInput").ap() \
        .rearrange("(c2 p) d -> p c2 d", p=128)
    wvT_d = nc.dram_tensor("w_vT", [C, DH], F32R, kind="ExternalInput").ap() \
        .rearrange("(c2 p) e -> p c2 e", p=128)
    wq_d = nc.dram_tensor("w_q", [DH, C], F32R, kind="ExternalInput").ap() \
        .rearrange("(d4 p) c -> p d4 c", p=128)
    wo_d = nc.dram_tensor("w_outT", [DH, C], F32R, kind="ExternalInput").ap() \
        .rearrange("(e4 p) o -> p e4 o", p=128)
    b_d = nc.dram_tensor("b_out", [C], F32, kind="ExternalInput").ap() \
        .rearrange("(o2 p) -> p o2", p=128)
    out_d = nc.dram_tensor("out", [C, n_tok], F16, kind="ExternalOutput").ap() \
        .rearrange("(o2 p) n -> p o2 n", p=128)

    def emit(tc, ctx):
        persist = ctx.enter_context(tc.tile_pool(name="persist", bufs=1))
        small = ctx.enter_context(tc.tile_pool(name="small", bufs=1))
        acc_ctx = ctx.enter_context(ExitStack())
        psacc = acc_ctx.enter_context(tc.tile_pool(name="psacc", bufs=1, space="PSUM"))

        img_cm_sb = persist.tile([128, 2, n_tok], F16)
        img_tm_sb = persist.tile([128, nta, CT], F8)
        wk_sb = persist.tile([128, 2, DH], F16)
        wvT_sb = persist.tile([128, 2, DH], F32R)
        wq_sb = persist.tile([128, 4, C], F32R)
        wo_sb = persist.tile([128, 4, C], F32R)
        b_sb = persist.tile([128, 2], F32)
        wcombT_sb = persist.tile([128, 2, C], F16)
        ident = persist.tile([128, 128], F32)

        chunks = [(0, 256), (256, 256)] + [
            (j * img_chunk, img_chunk) for j in range(1, n_tok // img_chunk)]

        def img_chunk_dma(c):
            # img_cm on the sync HWDGE ring; img_tm on the gpsimd (SWDGE) ring
            # so the two streams issue in parallel
            t0, nt = c
            sl = slice(t0, t0 + nt)
            nc.sync.dma_start(out=img_cm_sb[:, :, sl], in_=img_cm_d[:, :, sl])
            tsl = slice(t0 // TA, (t0 + nt) // TA)
            nc.gpsimd.dma_start(out=img_tm_sb[:, tsl, :], in_=img_tm_d[:, tsl, :])

        # first-needed data first: a small chunk 0 (it contends with everything
        # queued behind it, so small = early) + wk gate the first pass-A tile
        img_chunk_dma(chunks[0])
        nc.sync.dma_start(out=wk_sb, in_=wk_d)
        img_chunk_dma(chunks[1])
        masks.make_identity(nc, ident[:])

        # fill the initial DMA wait with dep-free matmuls: HAM un-throttles
        # (PE 1.2->2.4 GHz after ~3.4us busy) and the ACT exp table preloads
        wz = small.tile([128, 128], F16)
        nc.vector.memset(wz, 0.0)
        ebias = small.tile([128, 1], F32)
        nc.vector.memset(ebias, -2.0)
        dumm = small.tile([1, 8], F32)
        nc.vector.memset(dumm, 0.0)
        nc.scalar.activation(out=dumm, in_=dumm, func=AF.Exp)
        if warmup:
            with tc.tile_pool(name="warm", bufs=1, space="PSUM") as pw:
                wps = pw.tile([128, 128], F32)
                for _ in range(warmup):
                    nc.tensor.matmul(wps, lhsT=wz, rhs=wz, start=True, stop=True)

        # remaining img chunks, interleaving both layouts
        for c in chunks[2:]:
            img_chunk_dma(c)
        # fold-only weights issue after all img chunks: they are not needed
        # until the fold, and ahead of the chunks they would stall pass A
        nc.sync.dma_start(out=wvT_sb, in_=wvT_d)
        nc.sync.dma_start(out=wq_sb, in_=wq_d)
        nc.sync.dma_start(out=wo_sb, in_=wo_d)
        nc.sync.dma_start(out=b_sb, in_=b_d)

        # G accumulator [d-part (4 chunks), 257]: col 256 = S (ones column).
        # DR matmuls cost ~1 cycle per streamed OUTPUT column, so this
        # orientation (4x257 cols/macro) beats G^T + S (3x512 cols/macro).
        G_ps = psacc.tile([128, 4, DH], F32)

        # ---------------- PASS A (macro = 2 token tiles) ----------------
        nmac = nta // 2
        with ExitStack() as actx:
            pa = actx.enter_context(tc.tile_pool(name="pa", bufs=ek_bufs))
            psk = actx.enter_context(tc.tile_pool(name="psk", bufs=psk_bufs, space="PSUM"))
            eks = {}

            def emit_k(m):
                k_ps = psk.tile([128, 2, DH], F32, tag="kps")
                for j in range(2):
                    sl = slice((2 * m + j) * TA, (2 * m + j + 1) * TA)
                    for c2 in range(2):
                        nc.tensor.matmul(k_ps[:, j, :], lhsT=img_cm_sb[:, c2, sl],
                                         rhs=wk_sb[:, c2, :],
                                         start=(c2 == 0), stop=(c2 == 1))
                ek = pa.tile([128, 2, DH], F8, tag="ek")
                # exp(k - 2): constant logit shift keeps exp within fp8 range;
                # context = G/S is invariant to it
                nc.scalar.activation(out=ek, in_=k_ps, func=AF.Exp, bias=ebias[:, 0:1])
                eks[m] = ek

            def emit_g(m):
                # DoubleRow fp8: contracts both token tiles of the macro at once
                ek = eks.pop(m)
                st, sp = (m == 0), (m == nmac - 1)
                for dc in range(4):
                    nc.tensor.matmul(G_ps[:, dc, 0:257],
                                     lhsT=ek[:, :, dc * 128:(dc + 1) * 128],
                                     rhs=img_tm_sb[:, 2 * m:2 * m + 2, 0:257],
                                     perf_mode=DR,
                                     start=st, stop=sp, skip_group_check=True)

            for m in range(nmac):
                emit_k(m)
                if m >= 1:
                    emit_g(m - 1)
            emit_g(nmac - 1)

        # ---------------- FOLD ----------------
        with ExitStack() as wctx:
            G_sb = small.tile([128, 4, 257], F32)
            nc.vector.tensor_copy(out=G_sb, in_=G_ps[:, :, 0:257])
            scol = small.tile([128, 4], F32)
            nc.vector.tensor_copy(out=scol, in_=G_ps[:, :, 256:257])
            rs = small.tile([128, 4], F32)
            nc.vector.reciprocal(out=rs, in_=scol)
            acc_ctx.close()  # free G banks

            # dep-free filler MMs keep the PE's activity monitor from
            # re-throttling the clock (1/2 rate) across the fold's DVE handoffs
            pwarm = wctx.enter_context(tc.tile_pool(name="pwarm", bufs=1, space="PSUM"))
            wfill_ps = pwarm.tile([128, DH], F32)

            def warm_fill(n):
                for _ in range(n):
                    nc.tensor.matmul(wfill_ps, lhsT=wz, rhs=wk_sb[:, 0, :],
                                     start=True, stop=True, skip_group_check=True)

            warm_fill(fold_warm)
            Gt_sb = small.tile([128, 2, DH], F32R)
            with ExitStack() as tctx:
                pst = tctx.enter_context(tc.tile_pool(name="pst", bufs=1, space="PSUM"))
                Gt_ps2 = pst.tile([128, 2, DH], F32)
                for dc in range(4):
                    for c2 in range(2):
                        nc.tensor.transpose(Gt_ps2[:, c2, dc * 128:(dc + 1) * 128],
                                            G_sb[:, dc, c2 * 128:(c2 + 1) * 128],
                                            ident)
                for c2 in range(2):
                    nc.vector.tensor_copy(out=Gt_sb[:, c2, :], in_=Gt_ps2[:, c2, :])
            warm_fill(fold_warm)

            # ctx^T[e, d] = sum_c w_v[e, c] G[d, c]  (full cross; only the
            # per-head 64x64 blocks are used downstream)
            ctxT_sb = small.tile([128, 4, DH], F32R)
            with ExitStack() as cctx:
                psc = cctx.enter_context(tc.tile_pool(name="psc", bufs=1, space="PSUM"))
                ctxT_ps = psc.tile([128, 4, DH], F32)
                for ec in range(4):
                    for c2 in range(2):
                        nc.tensor.matmul(ctxT_ps[:, ec, :],
                                         lhsT=wvT_sb[:, c2, ec * 128:(ec + 1) * 128],
                                         rhs=Gt_sb[:, c2, :],
                                         start=(c2 == 0), stop=(c2 == 1))
                warm_fill(fold_warm)
                nc.vector.memset(ctxT_sb.bitcast(F32), 0.0)
                for h in range(HEADS):
                    ec, po = h // 2, (h % 2) * 64
                    nc.vector.tensor_copy(
                        out=ctxT_sb[po:po + 64, ec, h * 64:(h + 1) * 64],
                        in_=ctxT_ps[po:po + 64, ec, h * 64:(h + 1) * 64])

            # W_eff^T[d, o] = (1/S[d]) * sum_e ctxT[e, d] w_outT[e, o]
            # (off-head blocks of ctxT_sb are zero, so full-K contraction works)
            psw = wctx.enter_context(tc.tile_pool(name="psw", bufs=1, space="PSUM"))
            weff_ps = psw.tile([128, 4, C], F32)
            for dc in range(4):
                nc.tensor.matmul(weff_ps[:, dc, :],
                                 lhsT=ctxT_sb[:, dc, dc * 128:(dc + 1) * 128],
                                 rhs=wo_sb[:, dc, :], start=True, stop=True)
            warm_fill(fold_warm)
            weff_sb = small.tile([128, 4, C], F32R)
            for dc in range(4):
                nc.vector.tensor_scalar_mul(out=weff_sb[:, dc, :],
                                            in0=weff_ps[:, dc, :],
                                            scalar1=rs[:, dc:dc + 1])

            # W_comb^T[c, o] = sum_d w_q[d, c] W_eff^T[d, o]
            wc_ps = psw.tile([128, 2, C], F32)
            for c2 in range(2):
                for dc in range(4):
                    nc.tensor.matmul(wc_ps[:, c2, :],
                                     lhsT=wq_sb[:, dc, c2 * 128:(c2 + 1) * 128],
                                     rhs=weff_sb[:, dc, :],
                                     start=(dc == 0), stop=(dc == 3))
            for c2 in range(2):
                nc.vector.tensor_copy(out=wcombT_sb[:, c2, :], in_=wc_ps[:, c2, :])
            warm_fill(fold_warm)

        # ---------------- PASS B: out = W_comb @ img + b ----------------
        with ExitStack() as bctx:
            pb = bctx.enter_context(tc.tile_pool(name="pb", bufs=pb_bufs))
            pso = bctx.enter_context(tc.tile_pool(name="pso", bufs=pso_bufs, space="PSUM"))
            for i in range(ntb):
                sl = slice(i * tb, (i + 1) * tb)
                # one PSUM bank per (o2, half): banks recycle independently so
                # the next iteration's matmuls never wait on a whole-tile evict
                ops = [pso.tile([128, 512], F32, tag=f"op{j}", name=f"op{j}")
                       for j in range(4)]
                for o2 in range(2):
                    for h in range(2):
                        hsl = slice(i * tb + h * 512, i * tb + (h + 1) * 512)
                        for c2 in range(2):
                            nc.tensor.matmul(ops[o2 * 2 + h],
                                             lhsT=wcombT_sb[:, c2, o2 * 128:(o2 + 1) * 128],
                                             rhs=img_cm_sb[:, c2, hsl],
                                             start=(c2 == 0), stop=(c2 == 1))
                out_sb = pb.tile([128, 2, tb], F16, tag="o")
                # split PSUM eviction across ACT and DVE so neither trails PE
                for h in range(2):
                    osl = slice(h * 512, (h + 1) * 512)
                    nc.scalar.activation(out=out_sb[:, 0, osl], in_=ops[h],
                                         func=AF.Identity, bias=b_sb[:, 0:1])
                    nc.vector.tensor_scalar_add(out=out_sb[:, 1, osl],
                                                in0=ops[2 + h],
                                                scalar1=b_sb[:, 1:2])
                if i == ntb - 1:  # halve the last transfer to drain sooner
                    for h in range(2):
                        osl = slice(h * 512, (h + 1) * 512)
                        dsl = slice(i * tb + h * 512, i * tb + (h + 1) * 512)
                        nc.sync.dma_start(out=out_d[:, :, dsl],
                                          in_=out_sb[:, :, osl])
                else:
                    nc.sync.dma_start(out=out_d[:, :, sl], in_=out_sb)

    with tile.TileContext(nc) as tc:
        for _rep in range(repeat):
            with ExitStack() as ctx:
                emit(tc, ctx)
            if repeat > 1:
                tc.strict_bb_all_engine_barrier()

    nc.compile()
    return nc


def _prep_inputs(img, w_qkv, w_out, b_out, n_tok=N_TOK):
    import ml_dtypes
    f8 = ml_dtypes.float8_e4m3
    img16 = np.asarray(img, dtype=np.float16).reshape(B, C, n_tok)
    img_cm = np.ascontiguousarray(img16)
    img_tm = np.zeros((B, n_tok, CT), dtype=f8)
    img_tm[:, :, 0:C] = img16.transpose(0, 2, 1).astype(f8)
    img_tm[:, :, C] = 1.0
    w_qkv = np.asarray(w_qkv, dtype=np.float32)
    w_kT = np.ascontiguousarray(w_qkv[DH:2 * DH].T.astype(np.float16))   # [256, 512]
    w_vT = np.ascontiguousarray(w_qkv[2 * DH:3 * DH].T)                  # [256, 512]
    w_q = np.ascontiguousarray(w_qkv[0:DH])                              # [512, 256]
    w_outT = np.ascontiguousarray(np.asarray(w_out, dtype=np.float32).T)  # [512, 256]
    b = np.ascontiguousarray(np.asarray(b_out, dtype=np.float32))
    return [
        {"img_cm": img_cm[i], "img_tm": img_tm[i], "w_kT": w_kT, "w_vT": w_vT,
         "w_q": w_q, "w_outT": w_outT, "b_out": b}
        for i in range(N_CORES)
    ]


class _Exec:
    """Compile once, execute many times on the 8 cores via PJRT/shard_map."""

    def __init__(self, nc):
        import jax
        import concourse.mybir as mybir
        from jax.experimental.shard_map import shard_map
        from jax.sharding import Mesh, PartitionSpec, NamedSharding
        from concourse.bass2jax import _bass_exec_p, install_neuronx_cc_hook, partition_id_tensor

        install_neuronx_cc_hook()
        self.jax = jax
        in_names, out_names, out_avals = [], [], []
        partition_name = nc.partition_id_tensor.name if nc.partition_id_tensor else None
        for alloc in nc.m.functions[0].allocations:
            if not isinstance(alloc, mybir.MemoryLocationSet):
                continue
            name = alloc.memorylocations[0].name
            if alloc.kind == "ExternalInput":
                if name != partition_name:
                    in_names.append(name)
            elif alloc.kind == "ExternalOutput":
                out_names.append(name)
                out_avals.append(jax.core.ShapedArray(
                    tuple(alloc.tensor_shape), mybir.dt.np(alloc.dtype)))
        self.in_names, self.out_names, self.out_avals = in_names, out_names, out_avals
        n_params = len(in_names)
        all_in_names = in_names + out_names
        if partition_name is not None:
            all_in_names.append(partition_name)

        def _body(*args):
            operands = list(args)
            if partition_name is not None:
                operands.append(partition_id_tensor())
            return tuple(_bass_exec_p.bind(
                *operands,
                out_avals=tuple(out_avals),
                in_names=tuple(all_in_names),
                out_names=tuple(out_names),
                lowering_input_output_aliases=(),
                sim_require_finite=True,
                sim_require_nnan=True,
                nc=nc,
            ))

        devices = jax.devices()[:N_CORES]
        mesh = Mesh(np.asarray(devices), ("core",))
        self._body = _body
        self.mesh = mesh
        self.sharding = NamedSharding(mesh, PartitionSpec("core"))
        n_ops = n_params + len(out_names)
        self.fn = jax.jit(
            shard_map(_body, mesh=mesh,
                      in_specs=(PartitionSpec("core"),) * n_ops,
                      out_specs=(PartitionSpec("core"),) * len(out_names),
                      check_rep=False),
            keep_unused=True,
        )
        self.dev_zeros = [
            jax.device_put(np.zeros((N_CORES * a.shape[0], *a.shape[1:]), a.dtype),
                           self.sharding)
            for a in out_avals
        ]

    def stage(self, in_maps):
        concat = [
            np.concatenate([np.asarray(m[name]) for m in in_maps], axis=0)
            for name in self.in_names
        ]
        return [self.jax.device_put(a, self.sharding) for a in concat]

    def run(self, staged):
        outs = self.fn(*staged, *self.dev_zeros)
        self.jax.block_until_ready(outs)
        return outs

    def results(self, outs):
        per_core = []
        for c in range(N_CORES):
            per_core.append({
                name: np.asarray(outs[i]).reshape(N_CORES, *self.out_avals[i].shape)[c]
                for i, name in enumerate(self.out_names)
            })
        return per_core


_CACHE = {}


def _get_exec():
    if "exec" not in _CACHE:
        _CACHE["exec"] = _Exec(_build_program())
    return _CACHE["exec"]


def kernel(img, w_qkv, w_out, b_out):
    ex = _get_exec()
    staged = ex.stage(_prep_inputs(img, w_qkv, w_out, b_out))
    res = ex.results(ex.run(staged))
    out = np.stack([res[i]["out"] for i in range(N_CORES)]).astype(np.float32)
    return out.reshape(B, C, X, Y)
